# revision 1
# baseline (speedup 1.0000x reference)
"""Distributed brute-force KNN (retrieval) kernel for one TRN2 chip (8 NeuronCores).

Problem: queries [256,128] f32, candidates [500000,128] f32, identifiers [500000] i32,
k=100. Output: (values [256,100] f32 desc-sorted, ids [256,100] i32).

Strategy:
  - Shard candidates over N across the 8 cores (62500 each).
  - Per core: bf16 matmul (Q stationary, C^T shard streamed) -> PSUM score
    tiles [128q, 500c]. ScalarE copies each tile to SBUF f32; VectorE folds
    500->250->125 with pairwise max (each folded slot covers a group of 4
    candidates), then max/max_index extract the top-8 (value, slot) per
    125-slot window per query-half. Claims accumulate in SBUF, one DMA out.
  - Host: expand each claimed slot to its 4 candidates, rescore contenders
    exactly in f64, and validate: any window whose 8th claimed value (or a
    duplicated claimed slot) could still hide a top-k element is fully
    rescanned on host. Exactness never depends on device numerics.
"""
import numpy as np
import ml_dtypes

B = 256          # queries
N = 500000       # candidates
D = 128          # dim
NCORES = 8
NSH = N // NCORES          # 62500 real candidates per core
TILE = 500                 # candidates per psum tile
NTILES = 125               # 62500/500
NSHP = NTILES * TILE       # == NSH (no padding needed)
FOLD = 4                   # candidates per claimed slot (two pairwise folds)
SLOTS = TILE // FOLD       # 125 slots per tile window
CLAIM = NTILES * 8         # claimed entries per (core, query-half) window row

_CACHE = {}


def build(ntiles=NTILES, loops=1, variant="fold3"):
    """Build + compile the per-core Bass program. Returns the compiled Bacc."""
    import concourse.bass as bass
    import concourse.tile as tile
    from concourse import bacc, mybir

    bf16 = mybir.dt.bfloat16
    f32 = mybir.dt.float32
    u16 = mybir.dt.uint16
    Copy = mybir.ActivationFunctionType.Copy
    nsh = ntiles * TILE

    nc = bacc.Bacc("TRN2", debug=False)
    qt = nc.dram_tensor("qt", [D, B], bf16, kind="ExternalInput").ap()
    ct = nc.dram_tensor("ct", [D, nsh], bf16, kind="ExternalInput").ap()
    v8 = nc.dram_tensor("v8", [B, ntiles * 8], f32, kind="ExternalOutput").ap()
    i8 = nc.dram_tensor("i8", [B, ntiles * 8], u16, kind="ExternalOutput").ap()

    CHUNK = 4  # ct tiles per DMA
    with tile.TileContext(nc) as tc:
        with (
            tc.tile_pool(name="qpool", bufs=1) as qpool,
            tc.tile_pool(name="cpool", bufs=3) as cpool,
            tc.tile_pool(name="psum", bufs=8, space="PSUM") as pp,
            tc.tile_pool(name="fold", bufs=4) as fpool,
            tc.tile_pool(name="acc", bufs=1) as accp,
        ):
            qtile = qpool.tile([D, B], bf16)
            nc.sync.dma_start(qtile[:], qt[:])
            vacc = [
                accp.tile([128, ntiles * 8], f32, tag=f"vacc{h}", name=f"vacc{h}")
                for h in range(2)
            ]
            iacc = [
                accp.tile([128, ntiles * 8], u16, tag=f"iacc{h}", name=f"iacc{h}")
                for h in range(2)
            ]

            def body(_iv=None):
                for tt in range(0, ntiles, CHUNK):
                    nct = min(CHUNK, ntiles - tt)
                    ctile = cpool.tile([D, CHUNK * TILE], bf16, tag="ct", name="ctile")
                    nc.sync.dma_start(
                        ctile[:, 0 : nct * TILE],
                        ct[:, bass.ds(tt * TILE, nct * TILE)],
                    )
                    for j in range(nct):
                        t = tt + j
                        for h in range(2):
                            ps = pp.tile([128, TILE], f32, name="ps")
                            nc.tensor.matmul(
                                ps[:],
                                lhsT=qtile[:, bass.ds(h * 128, 128)],
                                rhs=ctile[:, bass.ds(j * TILE, TILE)],
                                start=True,
                                stop=True,
                            )
                            vout = vacc[h][:, bass.ds(t * 8, 8)]
                            iout = iacc[h][:, bass.ds(t * 8, 8)]
                            f0 = fpool.tile([128, TILE], f32, tag="f0", name="f0")
                            nc.scalar.activation(f0[:], ps[:], Copy)
                            f1 = fpool.tile([128, TILE // 2], f32, tag="f1", name="f1")
                            nc.vector.tensor_max(
                                f1[:], f0[:, bass.ds(0, 250)], f0[:, bass.ds(250, 250)]
                            )
                            f2 = fpool.tile([128, SLOTS], f32, tag="f2", name="f2")
                            nc.vector.tensor_max(
                                f2[:], f1[:, bass.ds(0, 125)], f1[:, bass.ds(125, 125)]
                            )
                            nc.vector.max(vout, f2[:])
                            nc.vector.max_index(iout, vout, f2[:])

            if loops == 1:
                body()
            else:
                with tc.For_i(0, loops, 1) as iv:
                    body(iv)

            for h in range(2):
                nc.sync.dma_start(v8[bass.ds(h * 128, 128), :], vacc[h][:])
                nc.sync.dma_start(i8[bass.ds(h * 128, 128), :], iacc[h][:])
    nc.compile()
    return nc


def _get_nc():
    if "nc" not in _CACHE:
        _CACHE["nc"] = build()
    return _CACHE["nc"]


def make_in_maps(queries, candidates):
    qt = np.ascontiguousarray(queries.T).astype(ml_dtypes.bfloat16)
    cb = candidates.astype(ml_dtypes.bfloat16)
    in_maps = []
    for c in range(NCORES):
        ct = np.zeros((D, NSHP), dtype=ml_dtypes.bfloat16)
        ct[:, :NSH] = cb[c * NSH : (c + 1) * NSH].T
        in_maps.append({"qt": qt, "ct": ct})
    return in_maps


def _device_claims(queries, candidates):
    """Run the 8-core SPMD kernel; return claimed (vals, slot base gidx) arrays."""
    from concourse.bass_utils import run_bass_kernel_spmd

    nc = _get_nc()
    in_maps = make_in_maps(queries, candidates)
    res = None
    for attempt in range(3):
        try:
            res = run_bass_kernel_spmd(nc, in_maps, core_ids=list(range(NCORES))).results
            break
        except Exception:
            if attempt == 2:
                raise
            import time as _time

            _time.sleep(2.0)
    assert res is not None
    v8 = np.stack([r["v8"] for r in res]).astype(np.float32)   # [8, B, CLAIM]
    i8 = np.stack([r["i8"] for r in res]).astype(np.int64)     # [8, B, CLAIM] slot in [0,SLOTS)
    # padded-local base index of the claimed slot (member m adds m*SLOTS):
    offs = (np.arange(CLAIM) // 8) * TILE
    lbase = i8 + offs[None, None, :]                           # local in [0, NSHP)
    return v8, i8, lbase


def _expand_local(lb):
    """Expand local slot bases [...] -> FOLD local member indices [..., FOLD]."""
    return lb[..., None] + (np.arange(FOLD) * SLOTS)[None, :]


def kernel(queries, candidates, identifiers, k):
    queries = np.asarray(queries, dtype=np.float32)
    candidates = np.asarray(candidates, dtype=np.float32)
    identifiers = np.asarray(identifiers)
    kk = int(k)

    v8, i8, lbase = _device_claims(queries, candidates)
    core_off = (np.arange(NCORES) * NSH)[:, None, None]

    # flatten claims to [B, NCORES*CLAIM]
    vals = v8.transpose(1, 0, 2).reshape(B, NCORES * CLAIM)
    lflat = lbase.transpose(1, 0, 2).reshape(B, NCORES * CLAIM)
    cflat = np.broadcast_to(
        np.arange(NCORES)[None, :, None], (B, NCORES, CLAIM)
    ).reshape(B, NCORES * CLAIM)

    q64 = queries.astype(np.float64)
    sigma = np.linalg.norm(queries, axis=1)

    def rescore_members(lb, cores, q):
        """lb: local slot bases [M], cores [M] -> exact scores + global ids."""
        mem = _expand_local(lb)                       # [M, FOLD] local padded idx
        valid = mem < NSH
        gl = mem + cores[:, None] * NSH               # global real idx (where valid)
        gl_f = np.where(valid, gl, 0)
        sv = candidates[gl_f].astype(np.float64) @ q64[q]
        sv = np.where(valid, sv, -np.inf)
        return sv.ravel(), np.where(valid, gl, -1).ravel()

    # --- preselect top-C claims per query, rescore their groups exactly ---
    C = max(2 * kk, kk + 64)
    part = np.argpartition(-vals, C, axis=1)[:, :C]
    vsel = np.take_along_axis(vals, part, 1)
    lsel = np.take_along_axis(lflat, part, 1)
    csel = np.take_along_axis(cflat, part, 1)
    mem = _expand_local(lsel)                          # [B, C, FOLD]
    valid = mem < NSH
    gsel = np.where(valid, mem + csel[..., None] * NSH, 0)
    se = np.einsum("qcd,qd->qc", candidates[gsel.reshape(B, -1)].astype(np.float64), q64)
    se = np.where(valid.reshape(B, -1), se, -np.inf)
    se_g = se.reshape(B, C, FOLD)
    # device claim error bound per query (claim ~ max over group's exact scores)
    gmax = se_g.max(2)
    finite = np.isfinite(gmax)
    delta = np.where(finite, np.abs(vsel - gmax), 0.0).max(1)
    margin = 4.0 * delta + 1e-3 * sigma

    vk = -np.partition(-se, kk - 1, axis=1)[:, kk - 1]
    thr = vk - margin

    pool_v = [se[q] for q in range(B)]
    pool_g = [np.where(valid, mem + csel[..., None] * NSH, -1)[q].ravel() for q in range(B)]

    # 1) any claimed entry above thr that wasn't rescored
    selmask = np.zeros(vals.shape, dtype=bool)
    np.put_along_axis(selmask, part, True, 1)
    need = (vals >= thr[:, None]) & ~selmask
    for q in np.nonzero(need.any(1))[0]:
        sv, gl = rescore_members(lflat[q, need[q]], cflat[q, need[q]], q)
        pool_v[q] = np.concatenate([pool_v[q], sv])
        pool_g[q] = np.concatenate([pool_g[q], gl])

    # 2) suspect windows: (a) 8th claimed value could hide an unclaimed slot,
    #    (b) duplicated claimed slot (f32/bf16 value tie collapsing groups)
    tmin = v8[:, :, 7::8]                              # [8, B, NTILES]
    sus = tmin >= (thr - margin)[None, :, None]
    iw = np.sort(i8.reshape(NCORES, B, NTILES, 8), axis=3)
    hasdup = (np.diff(iw, axis=3) == 0).any(3)         # [8, B, NTILES]
    vmax_w = v8[:, :, 0::8]
    sus |= hasdup & (vmax_w >= (thr - margin)[None, :, None])
    for q, c, t in zip(*np.nonzero(sus.transpose(1, 0, 2))):
        base = t * TILE
        hi = min(base + TILE, NSH)
        if hi <= base:
            continue
        gb = c * NSH + base
        sv = candidates[gb : c * NSH + hi].astype(np.float64) @ q64[q]
        g = np.arange(gb, c * NSH + hi, dtype=np.int64)
        pool_v[q] = np.concatenate([pool_v[q], sv])
        pool_g[q] = np.concatenate([pool_g[q], g])

    # --- final exact top-k per query (dedupe, desc value, index tiebreak) --
    out_v = np.empty((B, kk), np.float32)
    out_g = np.empty((B, kk), np.int64)
    for q in range(B):
        keep = pool_g[q] >= 0
        g, first = np.unique(pool_g[q][keep], return_index=True)
        v32 = pool_v[q][keep][first].astype(np.float32)
        assert v32.size >= kk
        order = np.lexsort((g, -v32))[:kk]
        out_v[q] = v32[order]
        out_g[q] = g[order]

    top_ids = identifiers[out_g]
    return out_v, top_ids



# revision 5
# speedup vs baseline: 3.8887x; 3.8887x over previous
"""Distributed brute-force KNN kernel for one TRN2 chip (8 NeuronCores).

Problem: queries [256,128] f32, candidates [500000,128] f32, identifiers
[500000] i32, k=100. Output: (values [256,100] f32 desc, ids [256,100] i32).

Device strategy (per core, candidates sharded N/8 = 62500, padded 64512):
  - bf16 matmul in 1024-col "quanta" (one query half each) -> PSUM f32
    (2 banks per quantum, 4 quanta in flight).
  - Per batch of 7 blocks: first NDB=2 blocks are "D" role -> DVE
    tensor_reduce(max) folds the psum quantum 1024->128 (FOLD=8) straight
    into the slot accumulator (slot j covers candidates 8j..8j+7).
    Remaining 5 are "A" role -> ScalarE copies psum -> bf16 staging; a
    batched DVE tensor_max chain folds 1024->512->256->128 (slot j covers
    j+128m, m<8). Roles keep DVE and Act balanced (~0.76 ns/score).
  - Slot maxima [128, 2*8064] bf16 DMA'd out; no top-k on device.
Host: threshold the slot stream, expand + rescore candidate groups exactly
in f64, iterate until provably complete, emit exact top-k (value desc,
index asc tiebreak). Exactness never depends on device numerics.
"""
import numpy as np
import ml_dtypes

B = 256          # queries
N = 500000       # candidates
D = 128          # dim
NCORES = 8
NSH = N // NCORES            # 62500 real candidates per core
QCOLS = 1024                 # candidate cols per matmul quantum
NBLOCKS = 63                 # 1024-col blocks per core
NSHP = NBLOCKS * QCOLS       # 64512 padded
FOLD = 8                     # candidates per slot
S = QCOLS // FOLD            # 128 slots per block
NSLOTS = NBLOCKS * S         # 8064 slots per (core, query)
BATCH = 7                    # blocks per chain flush
NDB = 2                      # leading "D"-role blocks per batch

_CACHE = {}


def block_role(blk):
    """'D' (tensor_reduce direct) or 'A' (act copy + chain) for a block."""
    return "D" if (blk % BATCH) < NDB else "A"


def build(loops=1, nblocks=NBLOCKS, batch=BATCH, ndb=NDB, chunk=BATCH,
          variant=None):
    """Build + compile the per-core Bass program."""
    import concourse.bass as bass
    import concourse.tile as tile
    from concourse import bacc, mybir

    bf16 = mybir.dt.bfloat16
    f32 = mybir.dt.float32
    Copy = mybir.ActivationFunctionType.Copy
    X = mybir.AxisListType.X
    MAX = mybir.AluOpType.max

    nshp = nblocks * QCOLS
    nslots = nblocks * S
    nab = batch - ndb
    assert nblocks % batch == 0 and batch % chunk == 0

    nc = bacc.Bacc("TRN2", debug=False)
    qt = nc.dram_tensor("qt", [D, B], bf16, kind="ExternalInput").ap()
    ct = nc.dram_tensor("ct", [D, nshp], bf16, kind="ExternalInput").ap()
    sv = nc.dram_tensor("sv", [128, 2 * nslots], bf16, kind="ExternalOutput").ap()

    with tile.TileContext(nc) as tc:
        with (
            tc.tile_pool(name="qpool", bufs=1) as qpool,
            tc.tile_pool(name="cpool", bufs=2) as cpool,
            tc.tile_pool(name="psum", bufs=4, space="PSUM") as pp,
            tc.tile_pool(name="stage", bufs=2) as sp,
            tc.tile_pool(name="acc", bufs=1) as accp,
        ):
            qtile = qpool.tile([D, B], bf16)
            nc.sync.dma_start(qtile[:], qt[:])
            vacc = accp.tile([128, 2 * nslots], bf16, tag="vacc", name="vacc")
            v4 = vacc.rearrange("p (h b s) -> p h b s", h=2, s=S)

            def body(_iv=None):
                for b0 in range(0, nblocks, batch):
                    bA = sp.tile([128, 2 * nab, 1024], bf16, tag="bA", name="bA")
                    iA = 0
                    ctile = None
                    for bb in range(batch):
                        blk = b0 + bb
                        role = "D" if bb < ndb else "A"
                        if bb % chunk == 0:
                            ctile = cpool.tile(
                                [D, chunk * QCOLS], bf16, tag="ct", name="ctile"
                            )
                            nc.sync.dma_start(
                                ctile[:],
                                ct[:, bass.ds(blk * QCOLS, chunk * QCOLS)],
                            )
                        rhs = ctile[:, bass.ds((bb % chunk) * QCOLS, QCOLS)]
                        for h in range(2):
                            ps = pp.tile([128, QCOLS], f32, name="ps")
                            for mh in range(2):
                                nc.tensor.matmul(
                                    ps[:, bass.ds(mh * 512, 512)],
                                    lhsT=qtile[:, bass.ds(h * 128, 128)],
                                    rhs=rhs[:, bass.ds(mh * 512, 512)],
                                    start=True,
                                    stop=True,
                                )
                            if role == "D":
                                ps3 = ps.rearrange("p (g w) -> p g w", w=FOLD)
                                nc.vector.tensor_reduce(
                                    v4[:, h, blk, :], ps3[:], axis=X, op=MAX
                                )
                            else:
                                nc.scalar.activation(bA[:, iA, :], ps[:], Copy)
                                iA += 1
                    # batched bf16 chain for A-role quanta (order: blk-major,
                    # h-minor => stage row 2*(bb-ndb)+h)
                    c1 = sp.tile([128, 2 * nab, 512], bf16, tag="c1", name="c1")
                    nc.vector.tensor_max(c1[:], bA[:, :, 0:512], bA[:, :, 512:1024])
                    w = sp.tile([128, 2 * nab, 256], bf16, tag="w", name="w")
                    nc.vector.tensor_max(w[:], c1[:, :, 0:256], c1[:, :, 256:512])
                    for h in range(2):
                        src = w[:, h: 2 * nab: 2, :]
                        dst = v4[:, h, bass.ds(b0 + ndb, nab), :]
                        nc.vector.tensor_max(dst, src[:, :, 0:128], src[:, :, 128:256])
                    for h in range(2):
                        nc.sync.dma_start(
                            sv[:, bass.ds(h * nslots + b0 * S, batch * S)],
                            vacc[:, bass.ds(h * nslots + b0 * S, batch * S)],
                        )

            if loops == 1:
                body()
            else:
                with tc.For_i(0, loops, 1) as iv:
                    body(iv)
    nc.compile()
    return nc


def _get_nc():
    if "nc" not in _CACHE:
        _CACHE["nc"] = build()
    return _CACHE["nc"]


def make_in_maps(queries, candidates):
    qt = np.ascontiguousarray(queries.T).astype(ml_dtypes.bfloat16)
    cb = candidates.astype(ml_dtypes.bfloat16)
    in_maps = []
    for c in range(NCORES):
        ct = np.zeros((D, NSHP), dtype=ml_dtypes.bfloat16)
        ct[:, :NSH] = cb[c * NSH: (c + 1) * NSH].T
        in_maps.append({"qt": qt, "ct": ct})
    return in_maps


def _device_slots(queries, candidates):
    """Run the 8-core SPMD kernel; return slot maxima [NCORES, B, NSLOTS] f32."""
    from concourse.bass_utils import run_bass_kernel_spmd

    nc = _get_nc()
    in_maps = make_in_maps(queries, candidates)
    res = None
    for attempt in range(3):
        try:
            res = run_bass_kernel_spmd(nc, in_maps, core_ids=list(range(NCORES))).results
            break
        except Exception:
            if attempt == 2:
                raise
            import time as _time
            _time.sleep(2.0)
    assert res is not None
    out = np.empty((NCORES, B, NSLOTS), np.float32)
    for c in range(NCORES):
        svc = np.asarray(res[c]["sv"]).astype(np.float32)
        out[c, :128] = svc[:, :NSLOTS]
        out[c, 128:] = svc[:, NSLOTS:]
    return out


# Slot membership depends on the block's role:
#   'D': slot j of block b -> local candidates b*QCOLS + 8*j + m, m<8
#   'A': slot j of block b -> local candidates b*QCOLS + j + 128*m, m<8
_ROLE_D = np.array([block_role(b) == "D" for b in range(NBLOCKS)])


def _slot_members(slot_ids):
    """Global slot ids [0, NCORES*NSLOTS) -> member candidate global indices
    [..., FOLD]; -1 where padded/invalid."""
    core = slot_ids // NSLOTS
    rem = slot_ids % NSLOTS
    blk = rem // S
    j = rem % S
    is_d = _ROLE_D[blk]
    m = np.arange(FOLD)
    mem_d = (blk * QCOLS + 8 * j)[..., None] + m[None, :]
    mem_a = (blk * QCOLS + j)[..., None] + (m * S)[None, :]
    mem = np.where(is_d[..., None], mem_d, mem_a)
    valid = mem < NSH
    gl = core[..., None] * NSH + np.minimum(mem, NSH - 1)
    return np.where(valid, gl, -1)


def kernel(queries, candidates, identifiers, k):
    queries = np.asarray(queries, dtype=np.float32)
    candidates = np.asarray(candidates, dtype=np.float32)
    identifiers = np.asarray(identifiers)
    kk = int(k)

    sv = _device_slots(queries, candidates)               # [8, B, NSLOTS]
    V = sv.transpose(1, 0, 2).reshape(B, NCORES * NSLOTS)
    TS = V.shape[1]
    q64 = queries.astype(np.float64)

    J0 = max(2 * kk, kk + 92)
    sel = np.argpartition(-V, J0, axis=1)[:, :J0]
    selmask = np.zeros((B, TS), bool)
    np.put_along_axis(selmask, sel, True, 1)

    pool_v = [None] * B
    pool_g = [None] * B
    gmax = np.full((B, TS), -np.inf, np.float32)

    def rescore(q, slots):
        mem = _slot_members(slots)
        valid = mem >= 0
        gl = np.where(valid, mem, 0)
        svx = candidates[gl.reshape(-1)].reshape(*gl.shape, D).astype(np.float64)
        sc = svx @ q64[q]
        sc = np.where(valid, sc, -np.inf)
        return sc, mem

    for q in range(B):
        sc, mem = rescore(q, sel[q])
        pool_v[q] = sc.ravel()
        pool_g[q] = mem.ravel()
        gmax[q, sel[q]] = sc.max(1)

    for _round in range(8):
        fin = np.isfinite(gmax) & selmask
        under = np.where(fin, gmax - np.where(fin, V, 0), 0.0)
        eps = max(float(under.max()), 0.0)
        margin = 4.0 * eps + 0.05
        vk = np.empty(B)
        for q in range(B):
            vk[q] = -np.partition(-pool_v[q], kk - 1)[kk - 1]
        need = (V >= (vk[:, None] - margin)) & ~selmask
        if not need.any():
            break
        for q in np.nonzero(need.any(1))[0]:
            slots = np.nonzero(need[q])[0]
            sc, mem = rescore(q, slots)
            pool_v[q] = np.concatenate([pool_v[q], sc.ravel()])
            pool_g[q] = np.concatenate([pool_g[q], mem.ravel()])
            gmax[q, slots] = sc.max(1)
        selmask |= need
    else:
        raise RuntimeError("slot rescoring did not converge")

    out_v = np.empty((B, kk), np.float32)
    out_g = np.empty((B, kk), np.int64)
    for q in range(B):
        keep = pool_g[q] >= 0
        g, first = np.unique(pool_g[q][keep], return_index=True)
        v = pool_v[q][keep][first].astype(np.float32)
        assert v.size >= kk
        order = np.lexsort((g, -v))[:kk]
        out_v[q] = v[order]
        out_g[q] = g[order]

    top_ids = identifiers[out_g]
    return out_v, top_ids
